# revision 34
# baseline (speedup 1.0000x reference)
"""Local softmax attention (GNN message passing) on 8 Trainium2 NeuronCores.

Math (per batch b, node n):
  q/k/v = x @ W{q,k,v}.T + b{q,k,v}              [N, 128], 8 heads x d=16
  scores[n,k,h] = sum_d q[n,h,d] * k[nbr(n,k),h,d] / sqrt(d)
  attn = softmax over k (32 neighbors)
  out[n,h,d] = sum_k attn[n,k,h] * v[nbr(n,k),h,d]

Sharding: 8 cores, each owning a 2048-node range (both batches).

Phase 1 (projections, ~105us, PE-bound): every core redundantly computes
k|v (bf16) for all nodes of both batches on the TensorEngine (x^T tiles
stationary, all-bf16 matmul) and writes the packed rows (k|v for BOTH
batches -> one 1KB DRAM row per node) to a private DRAM scratch in 1 MiB
group transfers; q for its own nodes stays in SBUF with the bq bias
added on VectorE (hidden under the PE wall).  bk cancels in the softmax;
bv is added exactly at the end (sum_k attn == 1).

Phase 2 (~23us per 128-node tile, DVE-bound, software-pipelined): the
SWDGE gather pulls each node's 32 packed rows -- 4 calls x 1024 idxs per
tile on rotating queues, issued two tiles ahead; each gathered row
serves both batch instances.  All DVE instructions fuse BOTH batches
(~150ns fixed overhead each, 2x bf16 mode): q*kg products, a 4-level
pairwise-add tree over d for the scores (last level fp32), attn*vg
(exp expanded over d by ScalarE so both operands stay dense), a 5-level
pairwise tree over k, the 1/Z scale and the bv add.

The per-tile emission order is chosen around the tile framework's
TAG-GRANULAR dependency tracking (a read of a rotating pool
conservatively orders after the latest *emitted* write to that tag and
vice versa):
  front(t):  prod, d-tree -> scores            (DVE)
  mid(t-1):  prod2, z, 1/z                     (DVE, frees gather slots)
  gather(t+2) issue                            (anchors on mid's prod2)
  exp(t), split in k-halves                    (ScalarE, parity tags)
  tail(t-1): k-tree, scale, bias, store        (DVE, fills the exp seam)
expx/scores use per-parity tags so tile t's exp never falsely orders
against tile t-1's readers; mid/tail surround the exp so the DVE always
has exp-independent work while ScalarE exponentiates.  Output is bf16
(converted to f32 on host).

SPMD: all 8 cores run the identical program; per-core variation is data
only (each core's x^T is permuted so its own 2048 nodes come first, and
gather indices are remapped into that row space).
"""

import os
import sys

sys.path.insert(0, "/opt/trn_rl_repo")

from contextlib import ExitStack

import numpy as np

import concourse.bacc as bacc
import concourse.bass as bass
import concourse.tile as tile
from concourse import mybir

HEADS = 8
P = 128
NCALL = 4          # gather calls per tile (1024 idxs each)
GT = 8             # phase-1 tiles per DMA group


class Cfg:
    def __init__(self, N=16384, K=32, C=128, n_cores=8, B=2):
        self.N, self.K, self.C, self.n_cores, self.B = N, K, C, n_cores, B
        self.N_own = N // n_cores
        self.n_all_tiles = N // P
        self.n_own_tiles = self.N_own // P
        self.d = C // HEADS


def _ap(base, dims):
    return bass.AP(tensor=base.tensor, offset=base.offset,
                   ap=[base.ap[0]] + [list(x) for x in dims])


def _off(base, elems):
    return bass.AP(tensor=base.tensor, offset=base.offset + elems,
                   ap=base.ap)


def build_nc(cfg: Cfg):
    N, K, C, B = cfg.N, cfg.K, cfg.C, cfg.B
    H3 = 3 * C
    R = 2 * B * C              # packed row elems (k|v per batch): 512
    f32, bf16, i16 = mybir.dt.float32, mybir.dt.bfloat16, mybir.dt.int16
    T_all, T_own = cfg.n_all_tiles, cfg.n_own_tiles
    d = cfg.d
    KB = K * B                 # 64
    KBC = K * B * C            # 8192 elems per phase-2 work tile
    CE = C + HEADS             # 136: v row + z stripe per (k,b)
    KBCE = K * B * CE          # 8704 elems in the extended prod2 tile
    ni_call = K * P // NCALL
    k_call = K // NCALL
    cols_call = ni_call // 16
    idx_cols = NCALL * cols_call
    n_grp = T_all // GT

    nc = bacc.Bacc("TRN2", target_bir_lowering=False, debug=False,
                   num_swdge_queues=4)

    xt = nc.dram_tensor("xt", [C, B * N], bf16, kind="ExternalInput")
    wqkv = nc.dram_tensor("wqkv", [C, H3], bf16, kind="ExternalInput")
    brow = nc.dram_tensor("brow", [C, H3], f32, kind="ExternalInput")
    idxw = nc.dram_tensor("idxw", [P, T_own * idx_cols], i16,
                          kind="ExternalInput")
    outp = nc.dram_tensor("out", [B * cfg.N_own, C], bf16,
                          kind="ExternalOutput")

    with tile.TileContext(nc) as tc, ExitStack() as ctx:
        const = ctx.enter_context(tc.tile_pool(name="const", bufs=1))
        psum = ctx.enter_context(tc.tile_pool(name="psum", bufs=4, space="PSUM"))
        dram = ctx.enter_context(tc.tile_pool(name="dram", bufs=1, space="DRAM"))
        # gath also serves as the phase-1 staging pool (xt loads + kv store
        # groups) -- phase-1 tiles rotate through the same 8KB slots the
        # phase-2 gather stream uses.
        gath = ctx.enter_context(tc.tile_pool(name="gath", bufs=12))
        worka = ctx.enter_context(tc.tile_pool(name="worka", bufs=2))
        workb = ctx.enter_context(tc.tile_pool(name="workb", bufs=2))
        small = ctx.enter_context(tc.tile_pool(name="small", bufs=1))
        smx = ctx.enter_context(tc.tile_pool(name="smx", bufs=2))

        # --- constants ---
        wqkv_sb = const.tile([C, H3], bf16)
        nc.sync.dma_start(out=wqkv_sb[:], in_=wqkv[:, :])
        bvrep_sb = const.tile([P, B * C], bf16)  # bv replicated (both batches)
        b0 = brow[0:1, 2 * C : 3 * C]
        nc.gpsimd.dma_start(
            out=bvrep_sb[:],
            in_=bass.AP(tensor=b0.tensor, offset=b0.offset,
                        ap=[[0, P], [0, B], [1, C]]))
        bqrep_sb = const.tile([P, C], f32)   # bq replicated across partitions
        q0 = brow[0:1, 0:C]
        nc.gpsimd.dma_start(
            out=bqrep_sb[:],
            in_=bass.AP(tensor=q0.tensor, offset=q0.offset, ap=[[0, P], [1, C]]))
        idx_sb = const.tile([P, T_own * idx_cols], i16)
        nc.sync.dma_start(out=idx_sb[:], in_=idxw[:, :])
        q_all = const.tile([P, T_own * B * C], bf16)   # [t][b][hd]

        kv_dram = dram.tile([N, R], bf16)

        # --- phase 1: projections (both batches, all nodes) ---
        for g in range(n_grp):
            xt_g = gath.tile([P, B, GT, P], bf16, tag="g8k")
            for b in range(B):
                nc.sync.dma_start(
                    out=xt_g[:, b],
                    in_=xt[:, b * N + g * GT * P : b * N + (g + 1) * GT * P])
            kv_g = gath.tile([P, GT, B, 2 * C], bf16, tag="g8k")
            for t8 in range(GT):
                t = g * GT + t8
                if t < T_own:
                    for b in range(B):
                        pt = psum.tile([P, H3], f32, bufs=3)
                        nc.tensor.matmul(out=pt[:], lhsT=xt_g[:, b, t8],
                                         rhs=wqkv_sb[:],
                                         start=True, stop=True)
                        q_slot = (t * B + b) * C
                        nc.vector.tensor_tensor(
                            out=q_all[:, q_slot : q_slot + C], in0=pt[:, 0:C],
                            in1=bqrep_sb[:], op=mybir.AluOpType.add)
                        nc.scalar.activation(
                            out=kv_g[:, t8, b], in_=pt[:, C:H3],
                            func=mybir.ActivationFunctionType.Copy)
                else:
                    ptb = psum.tile([P, B, 2 * C], f32, bufs=5)
                    for b in range(B):
                        nc.tensor.matmul(out=ptb[:, b],
                                         lhsT=xt_g[:, b, t8],
                                         rhs=wqkv_sb[:, C:H3],
                                         start=True, stop=True)
                    if t8 % 2 == 0:
                        nc.scalar.activation(
                            out=kv_g[:, t8], in_=ptb[:],
                            func=mybir.ActivationFunctionType.Copy)
                    else:
                        nc.vector.tensor_scalar_add(kv_g[:, t8], ptb[:], 0.0)
            dst = kv_dram[g * GT * P : (g + 1) * GT * P, :]
            nc.scalar.dma_start(
                out=bass.AP(tensor=dst.tensor, offset=dst.offset,
                            ap=[[R, P], [R * P, GT], [1, R]]),
                in_=kv_g[:])

        # --- phase 2: software-pipelined gather + attention ---
        # Gathers are issued two tiles ahead; each python iteration runs
        # the "front half" of tile t (q*kg products + score tree, then
        # hands scores to ScalarE for the exp) and the "back half" of
        # tile t-1 (attn*vg products, z, k-tree, scale, bias, store).
        # While ScalarE exponentiates tile t, the DVE is busy with tile
        # t-1's back half -- no cross-engine stall, and the gather slots
        # freed by tile t-1's prod2 reads keep the SWDGE stream running
        # two tiles ahead of consumption.
        ni_reg = nc.gpsimd.to_reg(ni_call)
        ni_reg_h = nc.gpsimd.to_reg(ni_call // 2)
        KBH = K * B * HEADS   # 512

        def issue_gather(t, split=1):
            # split=2 halves each call (8 x 512 idxs): used for the first
            # two tiles so tile-0 compute starts sooner after the phase-1
            # stores land (the ramp is latency-, not throughput-bound).
            # Pure idx-column slicing; sub-call i covers k-slots
            # [i*kc, (i+1)*kc) of the same layout.
            ncall = NCALL * split
            kc = k_call // split
            cc = cols_call // split
            reg = ni_reg if split == 1 else ni_reg_h
            kvgs = []
            for i in range(ncall):
                kvg_i = gath.tile([P, kc, R], bf16, tag="g8k")
                c0 = t * idx_cols + i * cc
                nc.gpsimd.dma_gather(
                    out_ap=kvg_i[:],
                    in_ap=kv_dram[:],
                    idxs_ap=idx_sb[:, c0 : c0 + cc],
                    num_idxs=ni_call // split,
                    num_idxs_reg=reg,
                    elem_size=R,
                    queue_num=i % 4,
                )
                kvgs.append(kvg_i)
            return kvgs

        def mid_half(t, kvgs, expx, scores):
            # Extended prod2 tile: each (k,b) row is [128 attn*v | 8 z]
            # where the z stripe holds the compact exp values, so the
            # k-tree below reduces the weighted sum AND the softmax
            # denominator in one pass (replaces a separate strided
            # z-reduce).  The stripe is written by a tiny ScalarE exp
            # (runs right after this tile's main exps / the previous
            # tail's reads).
            p2t = worka.tile([P, KBCE], bf16, tag="w16k")
            nc.scalar.activation(
                out=_ap(_off(p2t[:], C), [[CE, K * B], [1, HEADS]]),
                in_=_ap(scores[:], [[HEADS, K * B], [1, HEADS]]),
                func=mybir.ActivationFunctionType.Exp,
                scale=1.0 / float(np.sqrt(d)))
            # prod2 = expx * vg  (bf16 2x, both dense); reading the v
            # halves frees the gather slots for tile t+2's calls.
            kc = K // len(kvgs)
            for i, kv in enumerate(kvgs):
                nc.vector.tensor_tensor(
                    out=_ap(_off(p2t[:], i * kc * B * CE),
                            [[B * CE, kc], [CE, B], [1, C]]),
                    in0=_ap(_off(kv[:], C), [[R, kc], [2 * C, B], [1, C]]),
                    in1=_ap(_off(expx[:], i * kc * B * C),
                            [[B * C, kc], [C, B], [1, C]]),
                    op=mybir.AluOpType.mult)
            return p2t

        def tail_half(t, prod2, s4k, s2k, s1k, a05):
            # acc_e = sum_k [attn*v | exp] : 5-level pairwise tree over
            # k (flat halves of the extended tile); cols 128..135 of each
            # b-block land the softmax denominator z.
            nc.vector.tensor_tensor(
                out=_ap(s4k, [[1, KBCE // 2]]),
                in0=prod2[:, 0 : KBCE // 2],
                in1=prod2[:, KBCE // 2 : KBCE],
                op=mybir.AluOpType.add)
            nc.vector.tensor_tensor(
                out=_ap(s2k, [[1, KBCE // 4]]),
                in0=_ap(s4k, [[1, KBCE // 4]]),
                in1=_ap(_off(s4k, KBCE // 4), [[1, KBCE // 4]]),
                op=mybir.AluOpType.add)
            nc.vector.tensor_tensor(
                out=_ap(s1k, [[1, KBCE // 8]]),
                in0=_ap(s2k, [[1, KBCE // 8]]),
                in1=_ap(_off(s2k, KBCE // 8), [[1, KBCE // 8]]),
                op=mybir.AluOpType.add)
            nc.vector.tensor_tensor(
                out=_ap(a05, [[1, KBCE // 16]]),
                in0=_ap(s1k, [[1, KBCE // 16]]),
                in1=_ap(_off(s1k, KBCE // 16), [[1, KBCE // 16]]),
                op=mybir.AluOpType.add)
            acc_e = small.tile([P, B * CE], bf16)
            nc.vector.tensor_tensor(
                out=acc_e[:], in0=_ap(a05, [[1, B * CE]]),
                in1=_ap(_off(a05, B * CE), [[1, B * CE]]),
                op=mybir.AluOpType.add)

            # out = acc * (1/z) + bv   (bv exact: sum_k attn == 1)
            rz = small.tile([P, B * HEADS], bf16)
            with nc.allow_low_precision(reason="1/z in bf16; rel-err gate covers it"):
                nc.vector.reciprocal(
                    rz[:], _ap(_off(acc_e[:], C), [[CE, B], [1, HEADS]]))
            sc = small.tile([P, B * C], bf16)
            nc.vector.tensor_tensor(
                out=sc[:], in0=_ap(acc_e[:], [[CE, B], [1, C]]),
                in1=_ap(rz[:], [[HEADS, B], [1, HEADS], [0, d]]),
                op=mybir.AluOpType.mult)
            outt = smx.tile([P, B * C], bf16)
            nc.vector.tensor_tensor(
                out=outt[:], in0=sc[:], in1=bvrep_sb[:],
                op=mybir.AluOpType.add)
            dsto = outp[t * P : (t + 1) * P, :]
            nc.sync.dma_start(
                out=bass.AP(tensor=dsto.tensor, offset=dsto.offset,
                            ap=[[C, P], [cfg.N_own * C, B], [1, C]]),
                in_=outt[:])

        pend = {0: issue_gather(0, split=2)}
        if T_own > 1:
            pend[1] = issue_gather(1, split=2)
        back = None
        for t in range(T_own):
            kvgs = pend.pop(t)

            # scratch shared by tile t's d-tree and tile t-1's k-tree
            # (sequential lifetimes on the in-order DVE); a05 aliases the
            # tail of s4k (dead there by k-tree level 4).
            s4kt = small.tile([P, KBCE // 2], bf16)
            s2kt = small.tile([P, KBCE // 4], bf16)
            s1kt = small.tile([P, KBCE // 8], bf16)
            s4k, s2k, s1k = s4kt[:], s2kt[:], s1kt[:]
            a05 = _off(s4k, KBCE // 2 - KBCE // 16)

            # front(t): prod, d-tree, exp
            qt = q_all[:, t * B * C : (t + 1) * B * C]   # [b][hd]
            prod = worka.tile([P, KBC], bf16, tag="w16k")
            kc = K // len(kvgs)
            for i, kv in enumerate(kvgs):
                nc.vector.tensor_tensor(
                    out=_ap(_off(prod[:], i * kc * B * C),
                            [[B * C, kc], [C, B], [1, C]]),
                    in0=_ap(kv[:], [[R, kc], [2 * C, B], [1, C]]),
                    in1=_ap(qt, [[0, kc], [C, B], [1, C]]),
                    op=mybir.AluOpType.mult)

            nc.vector.tensor_tensor(
                out=_ap(s4k, [[8, KBH], [1, 8]]),
                in0=_ap(prod[:], [[d, KBH], [1, 8]]),
                in1=_ap(_off(prod[:], 8), [[d, KBH], [1, 8]]),
                op=mybir.AluOpType.add)
            nc.vector.tensor_tensor(
                out=_ap(s2k, [[4, KBH], [1, 4]]),
                in0=_ap(s4k, [[8, KBH], [1, 4]]),
                in1=_ap(_off(s4k, 4), [[8, KBH], [1, 4]]),
                op=mybir.AluOpType.add)
            nc.vector.tensor_tensor(
                out=_ap(s1k, [[2, KBH], [1, 2]]),
                in0=_ap(s2k, [[4, KBH], [1, 2]]),
                in1=_ap(_off(s2k, 2), [[4, KBH], [1, 2]]),
                op=mybir.AluOpType.add)
            scores = smx.tile([P, KBH], f32, tag=f"scores{t % 2}", bufs=1)   # (k, b, h)
            nc.vector.tensor_tensor(
                out=_ap(scores[:], [[1, KBH]]),
                in0=_ap(s1k, [[2, KBH]]),
                in1=_ap(_off(s1k, 1), [[2, KBH]]),
                op=mybir.AluOpType.add)

            # mid half of tile t-1 (prod2): anchored only on tile
            # t-1 events, so it interleaves with tile t's front half on
            # the DVE and frees the gather slots tile t+2 needs.
            if back is not None:
                bt, bkvgs, bexpx, bscores = back
                bprod2 = mid_half(bt, bkvgs, bexpx, bscores)
            if t + 2 < T_own:
                pend[t + 2] = issue_gather(t + 2)

            # expx[(k,b,hd)] = exp(scores/4) expanded over d (ScalarE).
            # Per-parity tile tags: the framework's dep tracking is
            # tag-granular, so without the split tile t's exp write would
            # falsely order after tile t-1's expx readers (and stall the
            # DVE for the whole exp).
            expx = workb.tile([P, KBC], bf16, tag=f"expx{t % 2}", bufs=1)
            for h2 in range(2):
                nc.scalar.activation(
                    out=_ap(_off(expx[:], h2 * (KBC // 2)),
                            [[C, K * B // 2], [d, HEADS], [1, d]]),
                    in_=_ap(_off(scores[:], h2 * (KBH // 2)),
                            [[HEADS, K * B // 2], [1, HEADS], [0, d]]),
                    func=mybir.ActivationFunctionType.Exp,
                    scale=1.0 / float(np.sqrt(d)))

            # tail of tile t-1 (k-tree, scale, bias, store): emitted
            # after the exp so this DVE work fills the seam while
            # ScalarE exponentiates tile t.
            if back is not None:
                tail_half(bt, bprod2, s4k, s2k, s1k, a05)
            back = (t, kvgs, expx, scores)
        bt, bkvgs, bexpx, bscores = back
        bprod2 = mid_half(bt, bkvgs, bexpx, bscores)
        tail_half(bt, bprod2, s4k, s2k, s1k, a05)

    nc.compile()
    return nc


def make_in_maps(cfg: Cfg, x, Wq, bq, Wk, bk, Wv, bv, neighbor_index):
    import ml_dtypes

    N, K, C, B = cfg.N, cfg.K, cfg.C, cfg.B
    T_own, N_own = cfg.n_own_tiles, cfg.N_own
    bf16 = ml_dtypes.bfloat16

    x = np.asarray(x, np.float32)
    wqkv = np.ascontiguousarray(np.concatenate(
        [np.asarray(Wq, np.float32).T, np.asarray(Wk, np.float32).T,
         np.asarray(Wv, np.float32).T], axis=1)).astype(bf16)
    brow = np.zeros((C, 3 * C), np.float32)
    brow[0, :] = np.concatenate(
        [np.asarray(bq, np.float32), np.asarray(bk, np.float32),
         np.asarray(bv, np.float32)])
    nbr = np.asarray(neighbor_index, np.int64)
    xtb = np.ascontiguousarray(x.transpose(0, 2, 1))   # [B, C, N]

    in_maps = []
    for c in range(cfg.n_cores):
        own = np.arange(c * N_own, (c + 1) * N_own)
        rest = np.concatenate(
            [np.arange(0, c * N_own), np.arange((c + 1) * N_own, N)])
        perm = np.concatenate([own, rest])
        inv = np.empty(N, np.int64)
        inv[perm] = np.arange(N)

        xt_c = np.ascontiguousarray(
            xtb[:, :, perm].transpose(1, 0, 2).reshape(C, B * N)).astype(bf16)

        nb = inv[nbr[own]]                                  # [N_own, K]
        vals = nb.reshape(T_own, P, K).transpose(0, 2, 1)   # [T, k, nl]
        vals = vals.reshape(T_own, NCALL, (K // NCALL) * P)
        a = vals.reshape(T_own, NCALL, (K // NCALL) * P // 16, 16)
        a = a.transpose(3, 0, 1, 2)                          # [16, T, NCALL, S]
        rep = np.tile(a, (8, 1, 1, 1))                       # [128, ...]
        idxw = np.ascontiguousarray(
            rep.reshape(P, T_own * (K * P // 16)).astype(np.int16))

        in_maps.append({
            "xt": xt_c, "wqkv": wqkv, "brow": brow, "idxw": idxw,
        })
    return in_maps


_CACHE = {}


def _get_nc(cfg: Cfg):
    key = (cfg.N, cfg.K, cfg.C, cfg.n_cores, cfg.B)
    if key not in _CACHE:
        _CACHE[key] = build_nc(cfg)
    return _CACHE[key]


def kernel(x, Wq, bq, Wk, bk, Wv, bv, neighbor_index, _trace=False):
    from concourse.bass_utils import run_bass_kernel_spmd

    x = np.asarray(x)
    B, N, C = x.shape
    K = np.asarray(neighbor_index).shape[1]
    cfg = Cfg(N=N, K=K, C=C, n_cores=8, B=B)
    nc = _get_nc(cfg)
    in_maps = make_in_maps(cfg, x, Wq, bq, Wk, bk, Wv, bv, neighbor_index)
    res = run_bass_kernel_spmd(nc, in_maps, core_ids=list(range(cfg.n_cores)),
                               trace=_trace)
    out = np.empty((B, N, C), np.float32)
    for c in range(cfg.n_cores):
        o = np.asarray(res.results[c]["out"], np.float32).reshape(B, cfg.N_own, C)
        out[:, c * cfg.N_own : (c + 1) * cfg.N_own, :] = o
    if _trace:
        kernel.last_results = res
    return out


# revision 35
# speedup vs baseline: 1.0335x; 1.0335x over previous
"""Local softmax attention (GNN message passing) on 8 Trainium2 NeuronCores.

Math (per batch b, node n):
  q/k/v = x @ W{q,k,v}.T + b{q,k,v}              [N, 128], 8 heads x d=16
  scores[n,k,h] = sum_d q[n,h,d] * k[nbr(n,k),h,d] / sqrt(d)
  attn = softmax over k (32 neighbors)
  out[n,h,d] = sum_k attn[n,k,h] * v[nbr(n,k),h,d]

Sharding: 8 cores, each owning a 2048-node range (both batches).

Phase 1 (projections, ~105us, PE-bound): every core redundantly computes
k|v (bf16) for all nodes of both batches on the TensorEngine (x^T tiles
stationary, all-bf16 matmul) and writes the packed rows (k|v for BOTH
batches -> one 1KB DRAM row per node) to a private DRAM scratch in 1 MiB
group transfers; q for its own nodes stays in SBUF with the bq bias
added on VectorE (hidden under the PE wall).  bk cancels in the softmax;
bv is added exactly at the end (sum_k attn == 1).

Phase 2 (~23us per 128-node tile, DVE-bound, software-pipelined): the
SWDGE gather pulls each node's 32 packed rows -- 4 calls x 1024 idxs per
tile on rotating queues, issued two tiles ahead; each gathered row
serves both batch instances.  All DVE instructions fuse BOTH batches
(~150ns fixed overhead each, 2x bf16 mode): q*kg products, a 4-level
pairwise-add tree over d for the scores (last level fp32), attn*vg
(exp expanded over d by ScalarE so both operands stay dense), a 5-level
pairwise tree over k, the 1/Z scale and the bv add.

The per-tile emission order is chosen around the tile framework's
TAG-GRANULAR dependency tracking (a read of a rotating pool
conservatively orders after the latest *emitted* write to that tag and
vice versa):
  front(t):  prod, d-tree -> scores            (DVE)
  mid(t-1):  prod2, z, 1/z                     (DVE, frees gather slots)
  gather(t+2) issue                            (anchors on mid's prod2)
  exp(t), split in k-halves                    (ScalarE, parity tags)
  tail(t-1): k-tree, scale, bias, store        (DVE, fills the exp seam)
expx/scores use per-parity tags so tile t's exp never falsely orders
against tile t-1's readers; mid/tail surround the exp so the DVE always
has exp-independent work while ScalarE exponentiates.  Output is bf16
(converted to f32 on host).

SPMD: all 8 cores run the identical program; per-core variation is data
only (each core's x^T is permuted so its own 2048 nodes come first, and
gather indices are remapped into that row space).
"""

import os
import sys

sys.path.insert(0, "/opt/trn_rl_repo")

from contextlib import ExitStack

import numpy as np

import concourse.bacc as bacc
import concourse.bass as bass
import concourse.tile as tile
from concourse import mybir

HEADS = 8
P = 128
NCALL = 4          # gather calls per tile (1024 idxs each)
GT = 8             # phase-1 tiles per DMA group


class Cfg:
    def __init__(self, N=16384, K=32, C=128, n_cores=8, B=2):
        self.N, self.K, self.C, self.n_cores, self.B = N, K, C, n_cores, B
        self.N_own = N // n_cores
        self.n_all_tiles = N // P
        self.n_own_tiles = self.N_own // P
        self.d = C // HEADS


def _ap(base, dims):
    return bass.AP(tensor=base.tensor, offset=base.offset,
                   ap=[base.ap[0]] + [list(x) for x in dims])


def _off(base, elems):
    return bass.AP(tensor=base.tensor, offset=base.offset + elems,
                   ap=base.ap)


def build_nc(cfg: Cfg):
    N, K, C, B = cfg.N, cfg.K, cfg.C, cfg.B
    H3 = 3 * C
    R = 2 * B * C              # packed row elems (k|v per batch): 512
    f32, bf16, i16 = mybir.dt.float32, mybir.dt.bfloat16, mybir.dt.int16
    T_all, T_own = cfg.n_all_tiles, cfg.n_own_tiles
    d = cfg.d
    KB = K * B                 # 64
    KBC = K * B * C            # 8192 elems per phase-2 work tile
    CE = C + HEADS             # 136: v row + z stripe per (k,b)
    KBCE = K * B * CE          # 8704 elems in the extended prod2 tile
    ni_call = K * P // NCALL
    k_call = K // NCALL
    cols_call = ni_call // 16
    idx_cols = NCALL * cols_call
    n_grp = T_all // GT

    nc = bacc.Bacc("TRN2", target_bir_lowering=False, debug=False,
                   num_swdge_queues=4)

    xt = nc.dram_tensor("xt", [C, B * N], bf16, kind="ExternalInput")
    wqkv = nc.dram_tensor("wqkv", [C, H3], bf16, kind="ExternalInput")
    brow = nc.dram_tensor("brow", [C, H3], f32, kind="ExternalInput")
    idxw = nc.dram_tensor("idxw", [P, T_own * idx_cols], i16,
                          kind="ExternalInput")
    outp = nc.dram_tensor("out", [B * cfg.N_own, C], bf16,
                          kind="ExternalOutput")

    with tile.TileContext(nc) as tc, ExitStack() as ctx:
        const = ctx.enter_context(tc.tile_pool(name="const", bufs=1))
        psum = ctx.enter_context(tc.tile_pool(name="psum", bufs=4, space="PSUM"))
        dram = ctx.enter_context(tc.tile_pool(name="dram", bufs=1, space="DRAM"))
        # gath also serves as the phase-1 staging pool (xt loads + kv store
        # groups) -- phase-1 tiles rotate through the same 8KB slots the
        # phase-2 gather stream uses.
        gath = ctx.enter_context(tc.tile_pool(name="gath", bufs=12))
        worka = ctx.enter_context(tc.tile_pool(name="worka", bufs=2))
        workb = ctx.enter_context(tc.tile_pool(name="workb", bufs=2))
        small = ctx.enter_context(tc.tile_pool(name="small", bufs=1))
        smx = ctx.enter_context(tc.tile_pool(name="smx", bufs=2))

        # --- constants ---
        wqkv_sb = const.tile([C, H3], bf16)
        nc.sync.dma_start(out=wqkv_sb[:], in_=wqkv[:, :])
        bvrep_sb = const.tile([P, B * C], bf16)  # bv replicated (both batches)
        b0 = brow[0:1, 2 * C : 3 * C]
        nc.gpsimd.dma_start(
            out=bvrep_sb[:],
            in_=bass.AP(tensor=b0.tensor, offset=b0.offset,
                        ap=[[0, P], [0, B], [1, C]]))
        bqrep_sb = const.tile([P, C], f32)   # bq replicated across partitions
        q0 = brow[0:1, 0:C]
        nc.gpsimd.dma_start(
            out=bqrep_sb[:],
            in_=bass.AP(tensor=q0.tensor, offset=q0.offset, ap=[[0, P], [1, C]]))
        idx_sb = const.tile([P, T_own * idx_cols], i16)
        nc.sync.dma_start(out=idx_sb[:], in_=idxw[:, :])
        q_all = const.tile([P, T_own * B * C], bf16)   # [t][b][hd]

        kv_dram = dram.tile([N, R], bf16)

        # --- phase 1: projections (both batches, all nodes) ---
        for g in range(n_grp):
            xt_g = gath.tile([P, B, GT, P], bf16, tag="g8k")
            for b in range(B):
                nc.sync.dma_start(
                    out=xt_g[:, b],
                    in_=xt[:, b * N + g * GT * P : b * N + (g + 1) * GT * P])
            kv_g = gath.tile([P, GT, B, 2 * C], bf16, tag="g8k")
            for t8 in range(GT):
                t = g * GT + t8
                if t < T_own:
                    for b in range(B):
                        pt = psum.tile([P, H3], f32, bufs=3)
                        nc.tensor.matmul(out=pt[:], lhsT=xt_g[:, b, t8],
                                         rhs=wqkv_sb[:],
                                         start=True, stop=True)
                        q_slot = (t * B + b) * C
                        nc.vector.tensor_tensor(
                            out=q_all[:, q_slot : q_slot + C], in0=pt[:, 0:C],
                            in1=bqrep_sb[:], op=mybir.AluOpType.add)
                        nc.scalar.activation(
                            out=kv_g[:, t8, b], in_=pt[:, C:H3],
                            func=mybir.ActivationFunctionType.Copy)
                else:
                    ptb = psum.tile([P, B, 2 * C], f32, bufs=5)
                    for b in range(B):
                        nc.tensor.matmul(out=ptb[:, b],
                                         lhsT=xt_g[:, b, t8],
                                         rhs=wqkv_sb[:, C:H3],
                                         start=True, stop=True)
                    if t8 % 2 == 0:
                        nc.scalar.activation(
                            out=kv_g[:, t8], in_=ptb[:],
                            func=mybir.ActivationFunctionType.Copy)
                    else:
                        nc.vector.tensor_scalar_add(kv_g[:, t8], ptb[:], 0.0)
            dst = kv_dram[g * GT * P : (g + 1) * GT * P, :]
            nc.scalar.dma_start(
                out=bass.AP(tensor=dst.tensor, offset=dst.offset,
                            ap=[[R, P], [R * P, GT], [1, R]]),
                in_=kv_g[:])

        # --- phase 2: software-pipelined gather + attention ---
        # Gathers are issued two tiles ahead; each python iteration runs
        # the "front half" of tile t (q*kg products + score tree, then
        # hands scores to ScalarE for the exp) and the "back half" of
        # tile t-1 (attn*vg products, z, k-tree, scale, bias, store).
        # While ScalarE exponentiates tile t, the DVE is busy with tile
        # t-1's back half -- no cross-engine stall, and the gather slots
        # freed by tile t-1's prod2 reads keep the SWDGE stream running
        # two tiles ahead of consumption.
        ni_reg = nc.gpsimd.to_reg(ni_call)
        KBH = K * B * HEADS   # 512

        def issue_gather(t):
            kvgs = []
            for i in range(NCALL):
                kvg_i = gath.tile([P, k_call, R], bf16, tag="g8k")
                c0 = t * idx_cols + i * cols_call
                nc.gpsimd.dma_gather(
                    out_ap=kvg_i[:],
                    in_ap=kv_dram[:],
                    idxs_ap=idx_sb[:, c0 : c0 + cols_call],
                    num_idxs=ni_call,
                    num_idxs_reg=ni_reg,
                    elem_size=R,
                    queue_num=(t * NCALL + i) % 4,
                )
                kvgs.append(kvg_i)
            return kvgs

        def mid_half(t, kvgs, expx, scores):
            # Extended prod2 tile: each (k,b) row is [128 attn*v | 8 z]
            # where the z stripe holds the compact exp values, so the
            # k-tree below reduces the weighted sum AND the softmax
            # denominator in one pass (replaces a separate strided
            # z-reduce).  The stripe is written by a tiny ScalarE exp
            # (runs right after this tile's main exps / the previous
            # tail's reads).
            p2t = worka.tile([P, KBCE], bf16, tag="w16k")
            nc.scalar.activation(
                out=_ap(_off(p2t[:], C), [[CE, K * B], [1, HEADS]]),
                in_=_ap(scores[:], [[HEADS, K * B], [1, HEADS]]),
                func=mybir.ActivationFunctionType.Exp,
                scale=1.0 / float(np.sqrt(d)))
            # prod2 = expx * vg  (bf16 2x, both dense); reading the v
            # halves frees the gather slots for tile t+2's calls.
            for i in range(NCALL):
                nc.vector.tensor_tensor(
                    out=_ap(_off(p2t[:], i * k_call * B * CE),
                            [[B * CE, k_call], [CE, B], [1, C]]),
                    in0=_ap(_off(kvgs[i][:], C), [[R, k_call], [2 * C, B], [1, C]]),
                    in1=_ap(_off(expx[:], i * k_call * B * C),
                            [[B * C, k_call], [C, B], [1, C]]),
                    op=mybir.AluOpType.mult)
            return p2t

        def tail_half(t, prod2, s4k, s2k, s1k, a05):
            # acc_e = sum_k [attn*v | exp] : 5-level pairwise tree over
            # k (flat halves of the extended tile); cols 128..135 of each
            # b-block land the softmax denominator z.
            nc.vector.tensor_tensor(
                out=_ap(s4k, [[1, KBCE // 2]]),
                in0=prod2[:, 0 : KBCE // 2],
                in1=prod2[:, KBCE // 2 : KBCE],
                op=mybir.AluOpType.add)
            nc.vector.tensor_tensor(
                out=_ap(s2k, [[1, KBCE // 4]]),
                in0=_ap(s4k, [[1, KBCE // 4]]),
                in1=_ap(_off(s4k, KBCE // 4), [[1, KBCE // 4]]),
                op=mybir.AluOpType.add)
            nc.vector.tensor_tensor(
                out=_ap(s1k, [[1, KBCE // 8]]),
                in0=_ap(s2k, [[1, KBCE // 8]]),
                in1=_ap(_off(s2k, KBCE // 8), [[1, KBCE // 8]]),
                op=mybir.AluOpType.add)
            nc.vector.tensor_tensor(
                out=_ap(a05, [[1, KBCE // 16]]),
                in0=_ap(s1k, [[1, KBCE // 16]]),
                in1=_ap(_off(s1k, KBCE // 16), [[1, KBCE // 16]]),
                op=mybir.AluOpType.add)
            acc_e = small.tile([P, B * CE], bf16)
            nc.vector.tensor_tensor(
                out=acc_e[:], in0=_ap(a05, [[1, B * CE]]),
                in1=_ap(_off(a05, B * CE), [[1, B * CE]]),
                op=mybir.AluOpType.add)

            # out = acc * (1/z) + bv   (bv exact: sum_k attn == 1)
            rz = small.tile([P, B * HEADS], bf16)
            with nc.allow_low_precision(reason="1/z in bf16; rel-err gate covers it"):
                nc.vector.reciprocal(
                    rz[:], _ap(_off(acc_e[:], C), [[CE, B], [1, HEADS]]))
            sc = small.tile([P, B * C], bf16)
            nc.vector.tensor_tensor(
                out=sc[:], in0=_ap(acc_e[:], [[CE, B], [1, C]]),
                in1=_ap(rz[:], [[HEADS, B], [1, HEADS], [0, d]]),
                op=mybir.AluOpType.mult)
            outt = smx.tile([P, B * C], bf16)
            nc.vector.tensor_tensor(
                out=outt[:], in0=sc[:], in1=bvrep_sb[:],
                op=mybir.AluOpType.add)
            dsto = outp[t * P : (t + 1) * P, :]
            nc.sync.dma_start(
                out=bass.AP(tensor=dsto.tensor, offset=dsto.offset,
                            ap=[[C, P], [cfg.N_own * C, B], [1, C]]),
                in_=outt[:])

        pend = {0: issue_gather(0)}
        if T_own > 1:
            pend[1] = issue_gather(1)
        back = None
        for t in range(T_own):
            kvgs = pend.pop(t)

            # scratch shared by tile t's d-tree and tile t-1's k-tree
            # (sequential lifetimes on the in-order DVE); a05 aliases the
            # tail of s4k (dead there by k-tree level 4).
            s4kt = small.tile([P, KBCE // 2], bf16)
            s2kt = small.tile([P, KBCE // 4], bf16)
            s1kt = small.tile([P, KBCE // 8], bf16)
            s4k, s2k, s1k = s4kt[:], s2kt[:], s1kt[:]
            a05 = _off(s4k, KBCE // 2 - KBCE // 16)

            # front(t): prod, d-tree, exp
            qt = q_all[:, t * B * C : (t + 1) * B * C]   # [b][hd]
            prod = worka.tile([P, KBC], bf16, tag="w16k")
            for i in range(NCALL):
                nc.vector.tensor_tensor(
                    out=_ap(_off(prod[:], i * k_call * B * C),
                            [[B * C, k_call], [C, B], [1, C]]),
                    in0=_ap(kvgs[i][:], [[R, k_call], [2 * C, B], [1, C]]),
                    in1=_ap(qt, [[0, k_call], [C, B], [1, C]]),
                    op=mybir.AluOpType.mult)

            nc.vector.tensor_tensor(
                out=_ap(s4k, [[8, KBH], [1, 8]]),
                in0=_ap(prod[:], [[d, KBH], [1, 8]]),
                in1=_ap(_off(prod[:], 8), [[d, KBH], [1, 8]]),
                op=mybir.AluOpType.add)
            nc.vector.tensor_tensor(
                out=_ap(s2k, [[4, KBH], [1, 4]]),
                in0=_ap(s4k, [[8, KBH], [1, 4]]),
                in1=_ap(_off(s4k, 4), [[8, KBH], [1, 4]]),
                op=mybir.AluOpType.add)
            nc.vector.tensor_tensor(
                out=_ap(s1k, [[2, KBH], [1, 2]]),
                in0=_ap(s2k, [[4, KBH], [1, 2]]),
                in1=_ap(_off(s2k, 2), [[4, KBH], [1, 2]]),
                op=mybir.AluOpType.add)
            scores = smx.tile([P, KBH], f32, tag=f"scores{t % 2}", bufs=1)   # (k, b, h)
            nc.vector.tensor_tensor(
                out=_ap(scores[:], [[1, KBH]]),
                in0=_ap(s1k, [[2, KBH]]),
                in1=_ap(_off(s1k, 1), [[2, KBH]]),
                op=mybir.AluOpType.add)

            # mid half of tile t-1 (prod2): anchored only on tile
            # t-1 events, so it interleaves with tile t's front half on
            # the DVE and frees the gather slots tile t+2 needs.
            if back is not None:
                bt, bkvgs, bexpx, bscores = back
                bprod2 = mid_half(bt, bkvgs, bexpx, bscores)
            if t + 2 < T_own:
                pend[t + 2] = issue_gather(t + 2)

            # expx[(k,b,hd)] = exp(scores/4) expanded over d (ScalarE).
            # Per-parity tile tags: the framework's dep tracking is
            # tag-granular, so without the split tile t's exp write would
            # falsely order after tile t-1's expx readers (and stall the
            # DVE for the whole exp).
            expx = workb.tile([P, KBC], bf16, tag=f"expx{t % 2}", bufs=1)
            for h2 in range(2):
                nc.scalar.activation(
                    out=_ap(_off(expx[:], h2 * (KBC // 2)),
                            [[C, K * B // 2], [d, HEADS], [1, d]]),
                    in_=_ap(_off(scores[:], h2 * (KBH // 2)),
                            [[HEADS, K * B // 2], [1, HEADS], [0, d]]),
                    func=mybir.ActivationFunctionType.Exp,
                    scale=1.0 / float(np.sqrt(d)))

            # tail of tile t-1 (k-tree, scale, bias, store): emitted
            # after the exp so this DVE work fills the seam while
            # ScalarE exponentiates tile t.
            if back is not None:
                tail_half(bt, bprod2, s4k, s2k, s1k, a05)
            back = (t, kvgs, expx, scores)
        bt, bkvgs, bexpx, bscores = back
        bprod2 = mid_half(bt, bkvgs, bexpx, bscores)
        tail_half(bt, bprod2, s4k, s2k, s1k, a05)

    nc.compile()
    return nc


def make_in_maps(cfg: Cfg, x, Wq, bq, Wk, bk, Wv, bv, neighbor_index):
    import ml_dtypes

    N, K, C, B = cfg.N, cfg.K, cfg.C, cfg.B
    T_own, N_own = cfg.n_own_tiles, cfg.N_own
    bf16 = ml_dtypes.bfloat16

    x = np.asarray(x, np.float32)
    wqkv = np.ascontiguousarray(np.concatenate(
        [np.asarray(Wq, np.float32).T, np.asarray(Wk, np.float32).T,
         np.asarray(Wv, np.float32).T], axis=1)).astype(bf16)
    brow = np.zeros((C, 3 * C), np.float32)
    brow[0, :] = np.concatenate(
        [np.asarray(bq, np.float32), np.asarray(bk, np.float32),
         np.asarray(bv, np.float32)])
    nbr = np.asarray(neighbor_index, np.int64)
    xtb = np.ascontiguousarray(x.transpose(0, 2, 1))   # [B, C, N]

    in_maps = []
    for c in range(cfg.n_cores):
        own = np.arange(c * N_own, (c + 1) * N_own)
        rest = np.concatenate(
            [np.arange(0, c * N_own), np.arange((c + 1) * N_own, N)])
        perm = np.concatenate([own, rest])
        inv = np.empty(N, np.int64)
        inv[perm] = np.arange(N)

        xt_c = np.ascontiguousarray(
            xtb[:, :, perm].transpose(1, 0, 2).reshape(C, B * N)).astype(bf16)

        nb = inv[nbr[own]]                                  # [N_own, K]
        vals = nb.reshape(T_own, P, K).transpose(0, 2, 1)   # [T, k, nl]
        vals = vals.reshape(T_own, NCALL, (K // NCALL) * P)
        a = vals.reshape(T_own, NCALL, (K // NCALL) * P // 16, 16)
        a = a.transpose(3, 0, 1, 2)                          # [16, T, NCALL, S]
        rep = np.tile(a, (8, 1, 1, 1))                       # [128, ...]
        idxw = np.ascontiguousarray(
            rep.reshape(P, T_own * (K * P // 16)).astype(np.int16))

        in_maps.append({
            "xt": xt_c, "wqkv": wqkv, "brow": brow, "idxw": idxw,
        })
    return in_maps


_CACHE = {}


def _get_nc(cfg: Cfg):
    key = (cfg.N, cfg.K, cfg.C, cfg.n_cores, cfg.B)
    if key not in _CACHE:
        _CACHE[key] = build_nc(cfg)
    return _CACHE[key]


def kernel(x, Wq, bq, Wk, bk, Wv, bv, neighbor_index, _trace=False):
    from concourse.bass_utils import run_bass_kernel_spmd

    x = np.asarray(x)
    B, N, C = x.shape
    K = np.asarray(neighbor_index).shape[1]
    cfg = Cfg(N=N, K=K, C=C, n_cores=8, B=B)
    nc = _get_nc(cfg)
    in_maps = make_in_maps(cfg, x, Wq, bq, Wk, bk, Wv, bv, neighbor_index)
    res = run_bass_kernel_spmd(nc, in_maps, core_ids=list(range(cfg.n_cores)),
                               trace=_trace)
    out = np.empty((B, N, C), np.float32)
    for c in range(cfg.n_cores):
        o = np.asarray(res.results[c]["out"], np.float32).reshape(B, cfg.N_own, C)
        out[:, c * cfg.N_own : (c + 1) * cfg.N_own, :] = o
    if _trace:
        kernel.last_results = res
    return out
